# revision 25
# baseline (speedup 1.0000x reference)
"""DGI (Deep Graph Infomax) forward kernel for 8 TRN2 NeuronCores.

Problem (all shapes hardcoded):
  seq1, seq2: [1, 8192, 128] f32   node features
  adj:        [1, 8192, 8192] f32  dense adjacency
  cc_label:   [8, 1024] i32        community partition (arange layout)
  W: [128,128], b: [128], Wb: [128,128], bb: [] f32
  out:        [1, 16384] f32       = concat(ret1, ret2)

Math per GCN branch: h = relu(adj @ (seq @ W) + b). We reassociate to
(adj @ seq) @ W so the big contraction uses natural-layout seq tiles as
the stationary operand and a host-transposed adj block as the moving
operand; everything then lives in "transposed" space (features on
partitions), where the community mean is a free-axis reduction and the
bilinear scores are a 1-column matmul.

Sharding: core k owns nodes [1024k, 1024k+1024) == community k (cc_label
is arange). Each core reads its adjT column block (32 MB), the full seqs
(8 MB, replicated), computes its 1024 scores per branch. No collectives.

Per-core device program:
  ZT[d, n]   = sum_m seq_s[m, d] * adjT[m, n]     (64 accumulating matmuls
                                                   per psum tile, fp32r)
  aggT[h, n] = sum_d W[d, h] * ZT[d, n]
  hT         = relu(aggT + b)     (+ free-axis accum -> community sum)
  c          = sigmoid(sum / 1024)                [128, 1]
  cw         = Wb @ c      (lhsT = Wb^T from host) [128, 1]
  sc_s[n]    = sum_h hT_s[h, n] * cw[h] + bb      [1, 1024] per branch
"""

import numpy as np

import concourse.bass as bass
import concourse.tile as tile
from concourse import bacc, mybir
from concourse.bass_utils import run_bass_kernel_spmd

N = 8192          # nodes
D = 128           # input feature dim
H = 128           # hidden dim
NC = 8            # communities / cores
CS = N // NC      # community size (nodes per core)
MT = N // 128     # number of 128-row m-tiles (64)
CHUNK = 512       # matmul moving free dim (psum bank width in fp32)
NCH = CS // CHUNK # n-chunks per core (2)

F32 = mybir.dt.float32
F16 = mybir.dt.float16
ADJ_SCALE = 256.0  # keeps fp16(adj*scale) in the normal range; undone in the relu


def _build_module() -> bass.Bass:
    nc = bacc.Bacc()

    adjt = nc.declare_dram_parameter("adjt", [128, MT, CS], F16, isOutput=False)
    seq1 = nc.declare_dram_parameter("seq1", [128, MT, D], F16, isOutput=False)
    seq2 = nc.declare_dram_parameter("seq2", [128, MT, D], F16, isOutput=False)
    w = nc.declare_dram_parameter("w", [D, H], F32, isOutput=False)
    wbt = nc.declare_dram_parameter("wbt", [H, H], F32, isOutput=False)
    bvec = nc.declare_dram_parameter("bvec", [H, 1], F32, isOutput=False)
    bbvec = nc.declare_dram_parameter("bbvec", [1, 1], F32, isOutput=False)
    out = nc.declare_dram_parameter("out", [2, CS], F32, isOutput=True)

    with tile.TileContext(nc) as tc:
        _emit(tc, adjt, seq1, seq2, w, wbt, bvec, bbvec, out)
    nc.finalize()
    return nc


def _emit(tc, adjt, seq1, seq2, w, wbt, bvec, bbvec, out):
    nc = tc.nc
    with (
        tc.tile_pool(name="singles", bufs=1) as singles,
        tc.tile_pool(name="adj_pool", bufs=3) as adj_pool,
        tc.tile_pool(name="misc", bufs=1) as misc,
        tc.tile_pool(name="psum", bufs=1, space="PSUM") as psum,
    ):
        w_sb = singles.tile([D, H], F32)
        nc.gpsimd.dma_start(out=w_sb, in_=w[:])
        wbt_sb = singles.tile([H, H], F32)
        nc.gpsimd.dma_start(out=wbt_sb, in_=wbt[:])
        b_sb = singles.tile([H, 1], F32)
        nc.gpsimd.dma_start(out=b_sb, in_=bvec[:])
        bb_sb = singles.tile([1, 1], F32)
        nc.gpsimd.dma_start(out=bb_sb, in_=bbvec[:])

        # Warmup adjacency group issued first on the scalar queue so the PE
        # can start while sync's first big group is in flight.
        WARM = 2
        adj_warm = adj_pool.tile([128, WARM, CS], F16, name="adj_warm", bufs=1)
        nc.scalar.dma_start(out=adj_warm, in_=adjt[:, 0:WARM, :])

        # Seqs staged in growing chunks so the first matmul isn't gated on
        # the full 4 MB.
        seq1_sb = singles.tile([128, MT, D], F16)
        seq2_sb = singles.tile([128, MT, D], F16)
        SEQ_CHUNKS = [4, 4, 8, 16, 16, 16]
        pos = 0
        for n in SEQ_CHUNKS:
            sl = slice(pos, pos + n)
            nc.scalar.dma_start(out=seq1_sb[:, sl, :], in_=seq1[:, sl, :])
            nc.scalar.dma_start(out=seq2_sb[:, sl, :], in_=seq2[:, sl, :])
            pos += n
        seq_sb = (seq1_sb, seq2_sb)

        HALF = MT // 2
        # Z accumulators split by m-half: first half banks 0-3, second 4-7.
        z_half = [
            [
                [psum.tile([128, CHUNK], F32, name=f"z_ps_{h}_{s}_{c}") for c in range(NCH)]
                for s in range(2)
            ]
            for h in range(2)
        ]
        zt_sb = [
            [
                [misc.tile([128, CHUNK], F32, name=f"zt_sb_{h}_{s}_{c}") for c in range(NCH)]
                for s in range(2)
            ]
            for h in range(2)
        ]
        h_sb = [
            [misc.tile([128, CHUNK], F32, name=f"h_sb_{s}_{c}") for c in range(NCH)]
            for s in range(2)
        ]
        csum = [misc.tile([H, 1], F32, name=f"csum_{c}") for c in range(NCH)]

        # (queue, n_tiles): tiny warmup group on the scalar queue lets the
        # PE start while sync's first big group is still in flight.
        ADJ_GROUPS = [("w", WARM), ("s", 6)] + [("s", 8)] * 7
        assert sum(n for _, n in ADJ_GROUPS) == MT

        def copy_ps(dst, src_ps, c):
            if c == 0:
                nc.vector.tensor_copy(out=dst, in_=src_ps)
            else:
                nc.scalar.activation(
                    out=dst, in_=src_ps, func=mybir.ActivationFunctionType.Copy
                )

        def emit_half1_copies():
            for s in range(2):
                for c in range(NCH):
                    copy_ps(zt_sb[0][s][c], z_half[0][s][c], c)

        def emit_half1_agg():
            # First-pass W-contraction into the (now free) first-half banks.
            for s in range(2):
                for c in range(NCH):
                    nc.tensor.matmul(
                        z_half[0][s][c], w_sb, zt_sb[0][s][c], start=True, stop=False
                    )

        t0 = 0
        for gi, (q, gn) in enumerate(ADJ_GROUPS):
            if q == "w":
                adj_sb = adj_warm
            else:
                adj_sb = adj_pool.tile([128, gn, CS], F16, name="adj_sb", tag="adj_sb", bufs=4)
                nc.sync.dma_start(out=adj_sb, in_=adjt[:, t0 : t0 + gn, :])
            for u in range(gn):
                t = t0 + u
                h = 0 if t < HALF else 1
                for s in range(2):
                    lhsT = seq_sb[s][:, t, :]
                    for c in range(NCH):
                        nc.tensor.matmul(
                            z_half[h][s][c],
                            lhsT,
                            adj_sb[:, u, c * CHUNK : (c + 1) * CHUNK],
                            start=(t % HALF == 0),
                            stop=(t % HALF == HALF - 1),
                        )
            t0 += gn
            if t0 - gn < HALF <= t0:
                emit_half1_copies()
            if t0 - gn < HALF + 16 <= t0:
                emit_half1_agg()

        # Tail: branch 0 (drives the sigmoid/cw chain) first; branch 1's
        # matmuls/relu fill the PE while scalar runs sigmoid.
        for c in range(NCH):
            copy_ps(zt_sb[1][0][c], z_half[1][0][c], c)
        for c in range(NCH):
            nc.tensor.matmul(
                z_half[0][0][c], w_sb, zt_sb[1][0][c], start=False, stop=True
            )
            nc.scalar.activation(
                out=h_sb[0][c],
                in_=z_half[0][0][c],
                func=mybir.ActivationFunctionType.Relu,
                bias=b_sb,
                scale=1.0 / ADJ_SCALE,
                accum_out=csum[c],
            )
        for c in range(NCH):
            copy_ps(zt_sb[1][1][c], z_half[1][1][c], c)

        csum_tot = misc.tile([H, 1], F32)
        nc.vector.tensor_add(out=csum_tot, in0=csum[0], in1=csum[1])
        c_sb = misc.tile([H, 1], F32)
        nc.scalar.activation(
            out=c_sb,
            in_=csum_tot,
            func=mybir.ActivationFunctionType.Sigmoid,
            scale=1.0 / CS,
        )

        for c in range(NCH):
            nc.tensor.matmul(
                z_half[0][1][c], w_sb, zt_sb[1][1][c], start=False, stop=True
            )
        cw_ps = z_half[1][0][0]
        nc.tensor.matmul(cw_ps[:, :1], wbt_sb, c_sb, start=True, stop=True)
        for c in range(NCH):
            nc.scalar.activation(
                out=h_sb[1][c],
                in_=z_half[0][1][c],
                func=mybir.ActivationFunctionType.Relu,
                bias=b_sb,
                scale=1.0 / ADJ_SCALE,
            )
        cw_sb = misc.tile([H, 1], F32)
        nc.vector.tensor_copy(out=cw_sb, in_=cw_ps[:, :1])

        out_sb = misc.tile([1, 2, CS], F32)
        sc_banks = [
            [z_half[1][0][1], z_half[1][1][0]],
            [z_half[1][1][1], z_half[0][0][0]],
        ]
        for s in range(2):
            for c in range(NCH):
                nc.tensor.matmul(
                    sc_banks[s][c][:1, :], cw_sb, h_sb[s][c], start=True, stop=True
                )
            for c in range(NCH):
                nc.vector.tensor_scalar_add(
                    out=out_sb[:, s, c * CHUNK : (c + 1) * CHUNK],
                    in0=sc_banks[s][c][:1, :],
                    scalar1=bb_sb,
                )
            nc.gpsimd.dma_start(
                out=out[s : s + 1, :].unsqueeze(0), in_=out_sb[:, s, :].unsqueeze(1)
            )


_MODULE_CACHE: list = []


def get_module() -> bass.Bass:
    if not _MODULE_CACHE:
        _MODULE_CACHE.append(_build_module())
    return _MODULE_CACHE[0]


def shard_inputs(inputs: dict) -> list[dict]:
    """Full inputs -> per-core input maps (row-block sharding of adjT)."""
    def tile_seq(s):
        s16 = np.asarray(s, np.float32)[0].astype(np.float16)  # [N, D]
        return np.ascontiguousarray(s16.reshape(MT, 128, D).transpose(1, 0, 2))

    seq1 = tile_seq(inputs["seq1"])
    seq2 = tile_seq(inputs["seq2"])
    adj16 = (np.asarray(inputs["adj"], np.float32)[0] * ADJ_SCALE).astype(np.float16)
    w = np.ascontiguousarray(np.asarray(inputs["W"], np.float32))
    wbt = np.ascontiguousarray(np.asarray(inputs["Wb"], np.float32).T)
    bvec = np.asarray(inputs["b"], np.float32).reshape(H, 1).copy()
    bbvec = np.asarray(inputs["bb"], np.float32).reshape(1, 1).copy()

    in_maps = []
    for k in range(NC):
        in_maps.append(
            {
                "adjt": np.ascontiguousarray(
                    adj16[k * CS : (k + 1) * CS, :].T.reshape(MT, 128, CS).transpose(1, 0, 2)
                ),
                "seq1": seq1,
                "seq2": seq2,
                "w": w,
                "wbt": wbt,
                "bvec": bvec,
                "bbvec": bbvec,
            }
        )
    return in_maps


def gather_output(core_outs: list[np.ndarray], cc_label: np.ndarray) -> np.ndarray:
    """Per-core [2, CS] score blocks -> full [1, 2N] output.

    Scatter through cc_label mirrors the reference's .at[flat].set: entry
    (community k, position j) is the score of node cc_label[k, j].
    """
    sc1 = np.concatenate([o[0] for o in core_outs]).astype(np.float32)
    sc2 = np.concatenate([o[1] for o in core_outs]).astype(np.float32)
    flat = np.asarray(cc_label).reshape(-1)
    ret1 = np.zeros(N, np.float32)
    ret2 = np.zeros(N, np.float32)
    ret1[flat] = sc1
    ret2[flat] = sc2
    return np.concatenate([ret1, ret2])[None, :]


def kernel(**inputs) -> np.ndarray:
    nc = get_module()
    in_maps = shard_inputs(inputs)
    res = run_bass_kernel_spmd(nc, in_maps, core_ids=list(range(NC)))
    core_outs = [res.results[k]["out"] for k in range(NC)]
    return gather_output(core_outs, inputs["cc_label"])


if __name__ == "__main__":
    nc = get_module()
    print("module built ok")
